# revision 43
# baseline (speedup 1.0000x reference)
"""Trainium2 Bass kernel for nn_CausalSelfAttention_2783138808334.

B=8, T=1024, C=64, n_head=1. Data-parallel over batch: one batch per
NeuronCore across 8 cores (weights/tables replicated), gathered on the host.

Per-core algorithm:
  qkv = x @ Wqkv.T + b (bias folded in via an augmented K=65 contraction with
  a host-provided ones row); causal attention with relative-position tables;
  y = (att @ v + attU @ embv) @ Wproj.T (+ bproj added on the host).

Relative attention is computed in TWO domains concurrently:
  s-domain:  att[t,s] = a1[t,s] + QE[t,t-s]      (QE = q @ embk.T)
  u-domain: attU[t,u] = a1[t,t-u] + QE[t,u]
so the two diagonal "skews" (of QE and of a1) are independent and overlap.
Each skew writes REVERSED rows to a DRAM scratch at pitch P1 and reads back
with partition step P1-1 (unit inner stride). Both matrices ride ONE scratch
row per t -- [qe-rev | -4000 gap | a1-rev | -4000 gap] -- so one write + one
read per tile covers both domains, and the prefilled -4000 gaps land exactly
on the causal-mask region (exp -> 0), eliminating all masking ops.

E / AU are transposed 128x128-blockwise on the TensorEngine (groups of 4
into one PSUM bank, one strided copy per group into the big ETB/AUTB tiles).
Tiles are processed in order 4,5,6,7,3,2,1,0 so a medium tile's skew round
trip completes first and bridges the PE from the score matmuls into the
transpose+value phase; value term k becomes ready right after tile k's
transposes. Score matmul pairs run concurrently on PE row-groups (0,0)/(64,0)
via tile_position. A warm-up burst flips the HAM clock gate to 8/8 early.
"""
import numpy as np
import ml_dtypes

import concourse.bass as bass
import concourse.bacc as bacc
import concourse.mybir as mybir
from concourse import masks
from concourse.ap import AP

F32 = mybir.dt.float32
BF = mybir.dt.bfloat16
T = 1024
C = 64
NT = 8
P1 = 4096       # skew scratch row pitch (elements)
SCALE = 0.125   # 1/sqrt(C)
N_WARM = 4     # PE warm-up matmuls (HAM needs ~3.4us of sustained activity)
EXP = mybir.ActivationFunctionType.Exp


def rev_free(ap):
    """Reverse the (contiguous) free dim of a 2D AP."""
    (ps, pc), (fs, fc) = ap.ap
    assert fs == 1, ap.ap
    return AP(ap.tensor, ap.offset + (fc - 1), [[ps, pc], [-1, fc]])


def mm_chunks(lo, hi, step=512):
    a = lo
    while a < hi:
        b = min(hi, (a // step + 1) * step)
        yield a, b
        a = b


def emit(nc, tc, xta_d, ekr_d, const_d, yd):
    MULT = mybir.AluOpType.mult
    ADD = mybir.AluOpType.add
    with (
        tc.tile_pool(name="const", bufs=1) as cp,
        tc.tile_pool(name="work", bufs=1) as wp,
        tc.tile_pool(name="psum", bufs=1, space="PSUM") as pp,
        tc.tile_pool(name="dram", bufs=1, space="DRAM") as dp,
    ):
        QAD = dp.tile([T + 1, P1], BF, name="QAD").tensor

        # ---- loads ----
        XTA = cp.tile([65, T], BF)      # [x.T ; ones]
        KNE = cp.tile([128, T], BF)     # rows 0:64 k.T (natural); 64:128 embk.T-rev
        CONST = cp.tile([128, 768], BF)
        FILLC = cp.tile([128, 1024], BF)
        nc.vector.memset(FILLC, -4000.0)
        ZROW = cp.tile([1, 512], BF)
        nc.vector.memset(ZROW, 0.0)
        # XTA first (it gates qk); prefills next on the SAME queue as the
        # skew reads (sync) so FIFO order guarantees prefill-before-read.
        nc.sync.dma_start(out=XTA[0:33, :], in_=xta_d[0:33, :])
        nc.scalar.dma_start(out=XTA[33:65, :], in_=xta_d[33:65, :])
        nc.sync.dma_start(
            out=AP(QAD, P1 + 128, [[P1, 128], [128 * P1 + 128, NT], [1, 128]]),
            in_=FILLC.rearrange("p (b c) -> p b c", b=NT))
        nc.sync.dma_start(
            out=AP(QAD, P1 + 384, [[P1, 128], [128 * P1 + 256, NT], [1, 128]]),
            in_=FILLC.rearrange("p (b c) -> p b c", b=NT))
        nc.scalar.dma_start(out=KNE[64:128, :], in_=ekr_d[:, :])
        nc.gpsimd.dma_start(out=CONST[:, :], in_=const_d[:, :])
        EMBV = CONST[:, 0:512]          # embv row-packed [p, 64n+c]
        WVA = CONST[0:65, 512:576]      # [Wv.T ; bv]
        WQKB = CONST[0:65, 576:704]     # [[Wq.T | Wk.T] ; [bq | bk]]
        WPT = CONST[0:64, 704:768]      # Wproj.T

        identb = cp.tile([128, 128], BF)
        masks.make_identity(nc, identb)

        # ---- PE warm-up burst (garbage matmuls, result never read) ----
        wu = pp.tile([128, 512], F32, tag="A", bufs=4, name="wu")
        for _ in range(N_WARM):
            nc.tensor.matmul(wu[:, :], FILLC[:, 0:128], FILLC[:, 0:512],
                             start=True, stop=True)

        # ---- qk projection: [q.T ; k.T] = [Wq.T|Wk.T ; bq|bk].T @ [x.T ; 1]
        QT2 = cp.tile([128, T], BF)     # q.T duplicated in both halves
        for a, b in mm_chunks(0, T):
            ps_qk = pp.tile([128, 512], F32, tag="A" if a == 0 else "B",
                            bufs=4, name="ps_qk")
            nc.tensor.matmul(ps_qk[:, 0:b - a], WQKB, XTA[:, a:b],
                             start=True, stop=True)
            nc.scalar.copy(QT2[0:64, a:b], ps_qk[0:64, 0:b - a])
            nc.vector.tensor_copy(QT2[64:128, a:b], ps_qk[0:64, 0:b - a])
            nc.vector.tensor_copy(KNE[0:64, a:b], ps_qk[64:128, 0:b - a])

        Zc = cp.tile([128, NT], F32)
        rz = cp.tile([128, NT], F32)

        # ---- stage 1: score matmuls (row-packed pairs), PSUM evac, skew ----
        qa_t = {}
        au2_t = {}

        def stage1(i):
            Wd = 128 * (i + 1)
            i0 = 128 * i
            qa = cp.tile([128, 2048], BF, tag=f"qa{i}", name=f"qa{i}")
            qa_t[i] = qa
            for a, b in mm_chunks(0, Wd):
                w = b - a
                ps_a1 = pp.tile([128, 512], F32, tag="A", bufs=4, name="ps_a1")
                ps_qe = pp.tile([128, 512], F32, tag="B", bufs=4, name="ps_qe")
                nc.tensor.matmul(ps_a1[:, 0:w], QT2[0:64, i0:i0 + 128],
                                 KNE[0:64, a:b],
                                 start=True, stop=True, tile_position=(0, 0))
                nc.tensor.matmul(ps_qe[:, 0:w], QT2[64:128, i0:i0 + 128],
                                 KNE[64:128, T - Wd + a:T - Wd + b],
                                 start=True, stop=True, tile_position=(64, 0))
                # qa row = [qe-rev (Wd) | a1-rev (Wd)]; DVE/ACT split
                nc.vector.tensor_copy(qa[:, a:b], ps_qe[:, 0:w])
                nc.scalar.copy(
                    rev_free(qa[:, 2 * Wd - b:2 * Wd - a]), ps_a1[:, 0:w])
                # skew write for this chunk: [qe-rev a:b | a1-rev mirrored]
                nc.gpsimd.dma_start(
                    out=AP(QAD, (i0 + 1) * P1 + a,
                           [[P1, 128], [2 * Wd + 128 - a - b, 2], [1, w]]),
                    in_=AP(qa[:, :].tensor, qa[:, :].offset + a,
                           [[qa[:, :].ap[0][0], 128], [2 * Wd - a - b, 2], [1, w]]))
            # merged skew read: a2 = [:, 0:Wd], a1U = [:, Wd+128:2Wd+128]
            L = 2 * Wd + 128
            au2 = cp.tile([128, 2304], BF, tag=f"au2{i}", name=f"au2{i}")
            au2_t[i] = au2
            nc.sync.dma_start(
                out=au2[:, 0:L],
                in_=AP(QAD, (i0 + 1) * P1 + 127, [[P1 - 1, 128], [1, L]]))

        V = cp.tile([128, 512], BF)     # v[128n+p, c] at [p, 64n+c]
        ETB = cp.tile([128, NT * 1024], BF, name="ETB")
        AUTB = cp.tile([128, NT * 1024], BF, name="AUTB")
        ETB3 = ETB.rearrange("p (k c) -> p k c", c=1024)
        AUTB3 = AUTB.rearrange("p (k c) -> p k c", c=1024)
        ps_y1 = pp.tile([C, 512], F32, tag="A", bufs=4, name="ps_y1")
        ps_y0 = pp.tile([C, 512], F32, tag="B", bufs=4, name="ps_y0")
        enau_t = {}

        def stage2(i):
            Wd = 128 * (i + 1)
            qa = qa_t[i]
            au2 = au2_t[i]
            s1 = wp.tile([128, 2048], BF, tag="s1", bufs=3)
            nc.vector.scalar_tensor_tensor(
                out=s1[:, 0:Wd], in0=rev_free(qa[:, Wd:2 * Wd]), scalar=1.0,
                in1=au2[:, 0:Wd], op0=MULT, op1=ADD)
            nc.vector.scalar_tensor_tensor(
                out=s1[:, 1024:1024 + Wd], in0=rev_free(qa[:, 0:Wd]),
                scalar=1.0, in1=au2[:, Wd + 128:2 * Wd + 128],
                op0=MULT, op1=ADD)
            enau = cp.tile([128, 2048], BF, tag=f"enau{i}", name=f"enau{i}")
            enau_t[i] = enau
            nc.scalar.activation(enau[:, 0:Wd], s1[:, 0:Wd], EXP, scale=SCALE,
                                 accum_out=Zc[:, i:i + 1])
            nc.scalar.activation(enau[:, 1024:1024 + Wd],
                                 s1[:, 1024:1024 + Wd], EXP, scale=SCALE)

        def stage3(i):
            i0 = 128 * i
            enau = enau_t[i]
            for half, dst3 in ((0, ETB3), (1, AUTB3)):
                for g in range(0, i + 1, 4):
                    gsz = min(4, i + 1 - g)
                    ps_t4 = pp.tile([128, 512], BF, tag="B", bufs=4,
                                    name="ps_t4")
                    for j in range(gsz):
                        k = g + j
                        nc.tensor.matmul(
                            ps_t4[:, 128 * j:128 * (j + 1)],
                            enau[:, 1024 * half + 128 * k:1024 * half + 128 * (k + 1)],
                            identb[:, :], is_transpose=True,
                            start=(j == 0), stop=(j == gsz - 1))
                    cpy = nc.vector.tensor_copy if (i + g // 4 + half) % 2 \
                        else nc.scalar.copy
                    cpy(dst3[:, g:g + gsz, i0:i0 + 128],
                        ps_t4[:, 0:128 * gsz].rearrange("p (k c) -> p k c", c=128))
            # value terms: processing order 4,5,6,7,3,2,1,0 -- after tile 7
            # all of k=7..4 are ready; smaller k become ready one per tile
            for k in ([7, 6, 5, 4] if i == 7 else [i] if i < 4 else []):
                k0 = 128 * k
                ta = max(512, k0)
                nc.tensor.matmul(ps_y1[:, ta - 512:512], V[:, C * k:C * (k + 1)],
                                 ETB[:, 1024 * k + ta:1024 * k + T],
                                 start=False, stop=False)
                nc.tensor.matmul(ps_y1[:, ta - 512:512],
                                 EMBV[:, C * k:C * (k + 1)],
                                 AUTB[:, 1024 * k + ta:1024 * k + T],
                                 start=False, stop=(k == 0))
                if k0 < 512:
                    nc.tensor.matmul(ps_y0[:, k0:512], V[:, C * k:C * (k + 1)],
                                     ETB[:, 1024 * k + k0:1024 * k + 512],
                                     start=False, stop=False)
                    nc.tensor.matmul(ps_y0[:, k0:512],
                                     EMBV[:, C * k:C * (k + 1)],
                                     AUTB[:, 1024 * k + k0:1024 * k + 512],
                                     start=False, stop=(k == 0))

        # ---- v4 driver: stage1 all, v-projs, then lag-1 stage2/stage3 ----
        ORDER = [4, 5, 6, 7, 3, 2, 1, 0]
        for i in ORDER:
            stage1(i)
        # v projection (PE filler during the first tiles' skew round trips)
        for n in range(NT):
            ps_v = pp.tile([128, C], F32, tag="A", bufs=4, name="ps_v")
            nc.tensor.matmul(ps_v[:, :], XTA[:, 128 * n:128 * (n + 1)], WVA,
                             start=True, stop=True)
            nc.vector.tensor_copy(V[:, C * n:C * (n + 1)], ps_v[:, :])
        nc.tensor.matmul(ps_y1[:, :], ZROW[:, 0:C], ZROW[:, :],
                         start=True, stop=False)
        nc.tensor.matmul(ps_y0[:, :], ZROW[:, 0:C], ZROW[:, :],
                         start=True, stop=False)
        prev = None
        for i in ORDER:
            stage2(i)
            if prev is not None:
                stage3(prev)
            prev = i
        nc.vector.reciprocal(rz[:, :], Zc[:, :])
        stage3(0)

        YSB = cp.tile([C, T], BF)
        nc.vector.tensor_copy(YSB[:, 512:1024], ps_y1[:, :])
        nc.vector.tensor_copy(YSB[:, 0:512], ps_y0[:, :])

        # ---- output projection + 1/Z; one batched output DMA ----
        YQ = cp.tile([128, 512], F32)
        for i in range(NT):
            ps_p = pp.tile([128, C], F32, tag="A", bufs=4, name="ps_p")
            nc.tensor.matmul(ps_p[:, :], YSB[:, 128 * i:128 * (i + 1)], WPT,
                             start=True, stop=True)
            nc.vector.tensor_scalar_mul(YQ[:, C * i:C * (i + 1)], ps_p[:, :],
                                        rz[:, i:i + 1])
        nc.gpsimd.dma_start(
            out=AP(yd.tensor, yd.offset, [[C, 128], [128 * C, NT], [1, C]]),
            in_=YQ.rearrange("p (i c) -> p i c", c=C))


_NC_CACHE = None


def _build():
    global _NC_CACHE
    if _NC_CACHE is not None:
        return _NC_CACHE
    nc = bacc.Bacc("TRN2", target_bir_lowering=False, debug=False)
    xta_d = nc.dram_tensor("xta", [65, T], BF, kind="ExternalInput")
    ekr_d = nc.dram_tensor("ekr", [C, T], BF, kind="ExternalInput")
    const_d = nc.dram_tensor("cpack", [128, 768], BF, kind="ExternalInput")
    yd = nc.dram_tensor("y", [T, C], F32, kind="ExternalOutput")
    from concourse.tile import TileContext
    with TileContext(nc) as tc:
        emit(nc, tc, xta_d.ap(), ekr_d.ap(), const_d.ap(), yd.ap())
    nc.compile()
    _NC_CACHE = nc
    return nc


def _prep(inputs):
    """Host-side packing of all device inputs (bf16, pre-transposed)."""
    bf = ml_dtypes.bfloat16
    x = np.asarray(inputs["x"], dtype=np.float32)
    Wqkv = np.asarray(inputs["Wqkv"], dtype=np.float32)
    bqkv = np.asarray(inputs["bqkv"], dtype=np.float32)
    embk = np.asarray(inputs["embk"], dtype=np.float32)
    embv = np.asarray(inputs["embv"], dtype=np.float32)
    Wproj = np.asarray(inputs["Wproj"], dtype=np.float32)

    B = x.shape[0]
    xta = np.empty((B, 65, T), dtype=bf)
    for b in range(B):
        xta[b, 0:64] = x[b].T.astype(bf)
        xta[b, 64] = 1.0
    ekr = np.ascontiguousarray(embk.T[:, ::-1]).astype(bf)

    const = np.zeros((128, 768), dtype=bf)
    const[:, 0:512] = embv.reshape(8, 128, 64).transpose(1, 0, 2).reshape(128, 512).astype(bf)
    const[0:64, 512:576] = Wqkv[128:192, :].T.astype(bf)
    const[64, 512:576] = bqkv[128:192].astype(bf)
    const[0:64, 576:704] = Wqkv[0:128, :].T.astype(bf)
    const[64, 576:704] = bqkv[0:128].astype(bf)
    const[0:64, 704:768] = Wproj.T.astype(bf)
    return xta, np.ascontiguousarray(ekr), np.ascontiguousarray(const)


def run_spmd(inputs, **kwargs):
    from concourse.bass_utils import run_bass_kernel_spmd
    x = np.asarray(inputs["x"], dtype=np.float32)
    B = x.shape[0]
    nc = _build()
    xta, ekr, const = _prep(inputs)
    in_maps = [dict(xta=np.ascontiguousarray(xta[b]), ekr=ekr, cpack=const)
               for b in range(B)]
    res = run_bass_kernel_spmd(nc, in_maps, core_ids=list(range(B)), **kwargs)
    bproj = np.asarray(inputs["bproj"], dtype=np.float32)
    y = np.stack([r["y"] for r in res.results], axis=0) + bproj[None, None, :]
    return y, res


def kernel(**inputs):
    y, _ = run_spmd(inputs)
    return y


# revision 44
# speedup vs baseline: 1.0294x; 1.0294x over previous
"""Trainium2 Bass kernel for nn_CausalSelfAttention_2783138808334.

B=8, T=1024, C=64, n_head=1. Data-parallel over batch: one batch per
NeuronCore across 8 cores (weights/tables replicated), gathered on the host.

Per-core algorithm:
  qkv = x @ Wqkv.T + b (bias folded in via an augmented K=65 contraction with
  a host-provided ones row); causal attention with relative-position tables;
  y = (att @ v + attU @ embv) @ Wproj.T (+ bproj added on the host).

Relative attention is computed in TWO domains concurrently:
  s-domain:  att[t,s] = a1[t,s] + QE[t,t-s]      (QE = q @ embk.T)
  u-domain: attU[t,u] = a1[t,t-u] + QE[t,u]
so the two diagonal "skews" (of QE and of a1) are independent and overlap.
Each skew writes REVERSED rows to a DRAM scratch at pitch P1 and reads back
with partition step P1-1 (unit inner stride). Both matrices ride ONE scratch
row per t -- [qe-rev | -4000 gap | a1-rev | -4000 gap] -- so one write + one
read per tile covers both domains, and the prefilled -4000 gaps land exactly
on the causal-mask region (exp -> 0), eliminating all masking ops.

E / AU are transposed 128x128-blockwise on the TensorEngine (groups of 4
into one PSUM bank, one strided copy per group into the big ETB/AUTB tiles).
Tiles are processed in order 4,5,6,7,3,2,1,0 so a medium tile's skew round
trip completes first and bridges the PE from the score matmuls into the
transpose+value phase; value term k becomes ready right after tile k's
transposes. Score matmul pairs run concurrently on PE row-groups (0,0)/(64,0)
via tile_position. A warm-up burst flips the HAM clock gate to 8/8 early.
"""
import numpy as np
import ml_dtypes

import concourse.bass as bass
import concourse.bacc as bacc
import concourse.mybir as mybir
from concourse import masks
from concourse.ap import AP

F32 = mybir.dt.float32
BF = mybir.dt.bfloat16
T = 1024
C = 64
NT = 8
P1 = 4096       # skew scratch row pitch (elements)
SCALE = 0.125   # 1/sqrt(C)
N_WARM = 4     # PE warm-up matmuls (HAM needs ~3.4us of sustained activity)
EXP = mybir.ActivationFunctionType.Exp


def rev_free(ap):
    """Reverse the (contiguous) free dim of a 2D AP."""
    (ps, pc), (fs, fc) = ap.ap
    assert fs == 1, ap.ap
    return AP(ap.tensor, ap.offset + (fc - 1), [[ps, pc], [-1, fc]])


def mm_chunks(lo, hi, step=512):
    a = lo
    while a < hi:
        b = min(hi, (a // step + 1) * step)
        yield a, b
        a = b


def emit(nc, tc, xta_d, ekr_d, const_d, yd):
    MULT = mybir.AluOpType.mult
    ADD = mybir.AluOpType.add
    with (
        tc.tile_pool(name="const", bufs=1) as cp,
        tc.tile_pool(name="work", bufs=1) as wp,
        tc.tile_pool(name="psum", bufs=1, space="PSUM") as pp,
        tc.tile_pool(name="dram", bufs=1, space="DRAM") as dp,
    ):
        QAD = dp.tile([T + 1, P1], BF, name="QAD").tensor

        # ---- loads ----
        XTA = cp.tile([65, T], BF)      # [x.T ; ones]
        KNE = cp.tile([128, T], BF)     # rows 0:64 k.T (natural); 64:128 embk.T-rev
        CONST = cp.tile([128, 768], BF)
        FILLC = cp.tile([128, 1024], BF)
        nc.vector.memset(FILLC, -4000.0)
        ZROW = cp.tile([1, 512], BF)
        nc.vector.memset(ZROW, 0.0)
        # XTA first (it gates qk); prefills next on the SAME queue as the
        # skew reads (sync) so FIFO order guarantees prefill-before-read.
        nc.sync.dma_start(out=XTA[0:33, :], in_=xta_d[0:33, :])
        nc.scalar.dma_start(out=XTA[33:65, :], in_=xta_d[33:65, :])
        nc.sync.dma_start(
            out=AP(QAD, P1 + 128, [[P1, 128], [128 * P1 + 128, NT], [1, 128]]),
            in_=FILLC.rearrange("p (b c) -> p b c", b=NT))
        nc.sync.dma_start(
            out=AP(QAD, P1 + 384, [[P1, 128], [128 * P1 + 256, NT], [1, 128]]),
            in_=FILLC.rearrange("p (b c) -> p b c", b=NT))
        nc.scalar.dma_start(out=KNE[64:128, :], in_=ekr_d[:, :])
        nc.gpsimd.dma_start(out=CONST[:, :], in_=const_d[:, :])
        EMBV = CONST[:, 0:512]          # embv row-packed [p, 64n+c]
        WVA = CONST[0:65, 512:576]      # [Wv.T ; bv]
        WQKB = CONST[0:65, 576:704]     # [[Wq.T | Wk.T] ; [bq | bk]]
        WPT = CONST[0:64, 704:768]      # Wproj.T

        identb = cp.tile([128, 128], BF)
        masks.make_identity(nc, identb)

        # ---- PE warm-up burst (garbage matmuls, result never read) ----
        wu = pp.tile([128, 512], F32, tag="A", bufs=4, name="wu")
        for _ in range(N_WARM):
            nc.tensor.matmul(wu[:, :], FILLC[:, 0:128], FILLC[:, 0:512],
                             start=True, stop=True)

        # ---- qk projection: [q.T ; k.T] = [Wq.T|Wk.T ; bq|bk].T @ [x.T ; 1]
        QT2 = cp.tile([128, T], BF)     # q.T duplicated in both halves
        for a, b in mm_chunks(0, T):
            ps_qk = pp.tile([128, 512], F32, tag="A" if a == 0 else "B",
                            bufs=4, name="ps_qk")
            nc.tensor.matmul(ps_qk[:, 0:b - a], WQKB, XTA[:, a:b],
                             start=True, stop=True)
            nc.scalar.copy(QT2[0:64, a:b], ps_qk[0:64, 0:b - a])
            nc.vector.tensor_copy(QT2[64:128, a:b], ps_qk[0:64, 0:b - a])
            nc.vector.tensor_copy(KNE[0:64, a:b], ps_qk[64:128, 0:b - a])

        Zc = cp.tile([128, NT], F32)
        rz = cp.tile([128, NT], F32)

        # ---- stage 1: score matmuls (row-packed pairs), PSUM evac, skew ----
        qa_t = {}
        au2_t = {}

        def stage1(i):
            Wd = 128 * (i + 1)
            i0 = 128 * i
            qa = cp.tile([128, 2048], BF, tag=f"qa{i}", name=f"qa{i}")
            qa_t[i] = qa
            for a, b in mm_chunks(0, Wd):
                w = b - a
                ps_a1 = pp.tile([128, 512], F32, tag="A", bufs=4, name="ps_a1")
                ps_qe = pp.tile([128, 512], F32, tag="B", bufs=4, name="ps_qe")
                nc.tensor.matmul(ps_a1[:, 0:w], QT2[0:64, i0:i0 + 128],
                                 KNE[0:64, a:b],
                                 start=True, stop=True, tile_position=(0, 0))
                nc.tensor.matmul(ps_qe[:, 0:w], QT2[64:128, i0:i0 + 128],
                                 KNE[64:128, T - Wd + a:T - Wd + b],
                                 start=True, stop=True, tile_position=(64, 0))
                # qa row = [qe-rev (Wd) | a1-rev (Wd)]; DVE/ACT split
                nc.vector.tensor_copy(qa[:, a:b], ps_qe[:, 0:w])
                nc.scalar.copy(
                    rev_free(qa[:, 2 * Wd - b:2 * Wd - a]), ps_a1[:, 0:w])
            for a, b in mm_chunks(0, Wd):
                w = b - a
                nc.gpsimd.dma_start(
                    out=AP(QAD, (i0 + 1) * P1 + a,
                           [[P1, 128], [2 * Wd + 128 - a - b, 2], [1, w]]),
                    in_=AP(qa[:, :].tensor, qa[:, :].offset + a,
                           [[qa[:, :].ap[0][0], 128], [2 * Wd - a - b, 2], [1, w]]))
            # merged skew read: a2 = [:, 0:Wd], a1U = [:, Wd+128:2Wd+128]
            L = 2 * Wd + 128
            au2 = cp.tile([128, 2304], BF, tag=f"au2{i}", name=f"au2{i}")
            au2_t[i] = au2
            nc.sync.dma_start(
                out=au2[:, 0:L],
                in_=AP(QAD, (i0 + 1) * P1 + 127, [[P1 - 1, 128], [1, L]]))

        V = cp.tile([128, 512], BF)     # v[128n+p, c] at [p, 64n+c]
        ETB = cp.tile([128, NT * 1024], BF, name="ETB")
        AUTB = cp.tile([128, NT * 1024], BF, name="AUTB")
        ETB3 = ETB.rearrange("p (k c) -> p k c", c=1024)
        AUTB3 = AUTB.rearrange("p (k c) -> p k c", c=1024)
        ps_y1 = pp.tile([C, 512], F32, tag="A", bufs=4, name="ps_y1")
        ps_y0 = pp.tile([C, 512], F32, tag="B", bufs=4, name="ps_y0")
        enau_t = {}

        def stage2(i):
            Wd = 128 * (i + 1)
            qa = qa_t[i]
            au2 = au2_t[i]
            s1 = wp.tile([128, 2048], BF, tag="s1", bufs=3)
            nc.vector.scalar_tensor_tensor(
                out=s1[:, 0:Wd], in0=rev_free(qa[:, Wd:2 * Wd]), scalar=1.0,
                in1=au2[:, 0:Wd], op0=MULT, op1=ADD)
            nc.vector.scalar_tensor_tensor(
                out=s1[:, 1024:1024 + Wd], in0=rev_free(qa[:, 0:Wd]),
                scalar=1.0, in1=au2[:, Wd + 128:2 * Wd + 128],
                op0=MULT, op1=ADD)
            enau = cp.tile([128, 2048], BF, tag=f"enau{i}", name=f"enau{i}")
            enau_t[i] = enau
            nc.scalar.activation(enau[:, 0:Wd], s1[:, 0:Wd], EXP, scale=SCALE,
                                 accum_out=Zc[:, i:i + 1])
            nc.scalar.activation(enau[:, 1024:1024 + Wd],
                                 s1[:, 1024:1024 + Wd], EXP, scale=SCALE)

        def stage3(i):
            i0 = 128 * i
            enau = enau_t[i]
            for half, dst3 in ((0, ETB3), (1, AUTB3)):
                for g in range(0, i + 1, 4):
                    gsz = min(4, i + 1 - g)
                    ps_t4 = pp.tile([128, 512], BF, tag="B", bufs=4,
                                    name="ps_t4")
                    for j in range(gsz):
                        k = g + j
                        nc.tensor.matmul(
                            ps_t4[:, 128 * j:128 * (j + 1)],
                            enau[:, 1024 * half + 128 * k:1024 * half + 128 * (k + 1)],
                            identb[:, :], is_transpose=True,
                            start=(j == 0), stop=(j == gsz - 1))
                    cpy = nc.vector.tensor_copy if (i + g // 4 + half) % 2 \
                        else nc.scalar.copy
                    cpy(dst3[:, g:g + gsz, i0:i0 + 128],
                        ps_t4[:, 0:128 * gsz].rearrange("p (k c) -> p k c", c=128))
            # value terms: processing order 4,5,6,7,3,2,1,0 -- after tile 7
            # all of k=7..4 are ready; smaller k become ready one per tile
            for k in ([7, 6, 5, 4] if i == 7 else [i] if i < 4 else []):
                k0 = 128 * k
                ta = max(512, k0)
                nc.tensor.matmul(ps_y1[:, ta - 512:512], V[:, C * k:C * (k + 1)],
                                 ETB[:, 1024 * k + ta:1024 * k + T],
                                 start=False, stop=False)
                nc.tensor.matmul(ps_y1[:, ta - 512:512],
                                 EMBV[:, C * k:C * (k + 1)],
                                 AUTB[:, 1024 * k + ta:1024 * k + T],
                                 start=False, stop=(k == 0))
                if k0 < 512:
                    nc.tensor.matmul(ps_y0[:, k0:512], V[:, C * k:C * (k + 1)],
                                     ETB[:, 1024 * k + k0:1024 * k + 512],
                                     start=False, stop=False)
                    nc.tensor.matmul(ps_y0[:, k0:512],
                                     EMBV[:, C * k:C * (k + 1)],
                                     AUTB[:, 1024 * k + k0:1024 * k + 512],
                                     start=False, stop=(k == 0))

        # ---- v4 driver: stage1 all, v-projs, then lag-1 stage2/stage3 ----
        ORDER = [4, 5, 6, 7, 3, 2, 1, 0]
        for i in ORDER:
            stage1(i)
        # v projection (PE filler during the first tiles' skew round trips)
        for n in range(NT):
            ps_v = pp.tile([128, C], F32, tag="A", bufs=4, name="ps_v")
            nc.tensor.matmul(ps_v[:, :], XTA[:, 128 * n:128 * (n + 1)], WVA,
                             start=True, stop=True)
            nc.vector.tensor_copy(V[:, C * n:C * (n + 1)], ps_v[:, :])
        nc.tensor.matmul(ps_y1[:, :], ZROW[:, 0:C], ZROW[:, :],
                         start=True, stop=False)
        nc.tensor.matmul(ps_y0[:, :], ZROW[:, 0:C], ZROW[:, :],
                         start=True, stop=False)
        prev = None
        for i in ORDER:
            stage2(i)
            if prev is not None:
                stage3(prev)
            prev = i
        nc.vector.reciprocal(rz[:, :], Zc[:, :])
        stage3(0)

        YSB = cp.tile([C, T], BF)
        nc.vector.tensor_copy(YSB[:, 512:1024], ps_y1[:, :])
        nc.vector.tensor_copy(YSB[:, 0:512], ps_y0[:, :])

        # ---- output projection + 1/Z; one batched output DMA ----
        YQ = cp.tile([128, 512], F32)
        for i in range(NT):
            ps_p = pp.tile([128, C], F32, tag="A", bufs=4, name="ps_p")
            nc.tensor.matmul(ps_p[:, :], YSB[:, 128 * i:128 * (i + 1)], WPT,
                             start=True, stop=True)
            nc.vector.tensor_scalar_mul(YQ[:, C * i:C * (i + 1)], ps_p[:, :],
                                        rz[:, i:i + 1])
        nc.gpsimd.dma_start(
            out=AP(yd.tensor, yd.offset, [[C, 128], [128 * C, NT], [1, C]]),
            in_=YQ.rearrange("p (i c) -> p i c", c=C))


_NC_CACHE = None


def _build():
    global _NC_CACHE
    if _NC_CACHE is not None:
        return _NC_CACHE
    nc = bacc.Bacc("TRN2", target_bir_lowering=False, debug=False)
    xta_d = nc.dram_tensor("xta", [65, T], BF, kind="ExternalInput")
    ekr_d = nc.dram_tensor("ekr", [C, T], BF, kind="ExternalInput")
    const_d = nc.dram_tensor("cpack", [128, 768], BF, kind="ExternalInput")
    yd = nc.dram_tensor("y", [T, C], F32, kind="ExternalOutput")
    from concourse.tile import TileContext
    with TileContext(nc) as tc:
        emit(nc, tc, xta_d.ap(), ekr_d.ap(), const_d.ap(), yd.ap())
    nc.compile()
    _NC_CACHE = nc
    return nc


def _prep(inputs):
    """Host-side packing of all device inputs (bf16, pre-transposed)."""
    bf = ml_dtypes.bfloat16
    x = np.asarray(inputs["x"], dtype=np.float32)
    Wqkv = np.asarray(inputs["Wqkv"], dtype=np.float32)
    bqkv = np.asarray(inputs["bqkv"], dtype=np.float32)
    embk = np.asarray(inputs["embk"], dtype=np.float32)
    embv = np.asarray(inputs["embv"], dtype=np.float32)
    Wproj = np.asarray(inputs["Wproj"], dtype=np.float32)

    B = x.shape[0]
    xta = np.empty((B, 65, T), dtype=bf)
    for b in range(B):
        xta[b, 0:64] = x[b].T.astype(bf)
        xta[b, 64] = 1.0
    ekr = np.ascontiguousarray(embk.T[:, ::-1]).astype(bf)

    const = np.zeros((128, 768), dtype=bf)
    const[:, 0:512] = embv.reshape(8, 128, 64).transpose(1, 0, 2).reshape(128, 512).astype(bf)
    const[0:64, 512:576] = Wqkv[128:192, :].T.astype(bf)
    const[64, 512:576] = bqkv[128:192].astype(bf)
    const[0:64, 576:704] = Wqkv[0:128, :].T.astype(bf)
    const[64, 576:704] = bqkv[0:128].astype(bf)
    const[0:64, 704:768] = Wproj.T.astype(bf)
    return xta, np.ascontiguousarray(ekr), np.ascontiguousarray(const)


def run_spmd(inputs, **kwargs):
    from concourse.bass_utils import run_bass_kernel_spmd
    x = np.asarray(inputs["x"], dtype=np.float32)
    B = x.shape[0]
    nc = _build()
    xta, ekr, const = _prep(inputs)
    in_maps = [dict(xta=np.ascontiguousarray(xta[b]), ekr=ekr, cpack=const)
               for b in range(B)]
    res = run_bass_kernel_spmd(nc, in_maps, core_ids=list(range(B)), **kwargs)
    bproj = np.asarray(inputs["bproj"], dtype=np.float32)
    y = np.stack([r["y"] for r in res.results], axis=0) + bproj[None, None, :]
    return y, res


def kernel(**inputs):
    y, _ = run_spmd(inputs)
    return y


# revision 45
# speedup vs baseline: 1.0297x; 1.0002x over previous
"""Trainium2 Bass kernel for nn_CausalSelfAttention_2783138808334.

B=8, T=1024, C=64, n_head=1. Data-parallel over batch: one batch per
NeuronCore across 8 cores (weights/tables replicated), gathered on the host.

Per-core algorithm:
  qkv = x @ Wqkv.T + b (bias folded in via an augmented K=65 contraction with
  a host-provided ones row); causal attention with relative-position tables;
  y = (att @ v + attU @ embv) @ Wproj.T (+ bproj added on the host).

Relative attention is computed in TWO domains concurrently:
  s-domain:  att[t,s] = a1[t,s] + QE[t,t-s]      (QE = q @ embk.T)
  u-domain: attU[t,u] = a1[t,t-u] + QE[t,u]
so the two diagonal "skews" (of QE and of a1) are independent and overlap.
Each skew writes REVERSED rows to a DRAM scratch at pitch P1 and reads back
with partition step P1-1 (unit inner stride). Both matrices ride ONE scratch
row per t -- [qe-rev | -4000 gap | a1-rev | -4000 gap] -- so one write + one
read per tile covers both domains, and the prefilled -4000 gaps land exactly
on the causal-mask region (exp -> 0), eliminating all masking ops.

E / AU are transposed 128x128-blockwise on the TensorEngine (groups of 4
into one PSUM bank, one strided copy per group into the big ETB/AUTB tiles).
Tiles are processed in order 4,5,6,7,3,2,1,0 so a medium tile's skew round
trip completes first and bridges the PE from the score matmuls into the
transpose+value phase; value term k becomes ready right after tile k's
transposes. Score matmul pairs run concurrently on PE row-groups (0,0)/(64,0)
via tile_position. A warm-up burst flips the HAM clock gate to 8/8 early.
"""
import numpy as np
import ml_dtypes

import concourse.bass as bass
import concourse.bacc as bacc
import concourse.mybir as mybir
from concourse import masks
from concourse.ap import AP

F32 = mybir.dt.float32
BF = mybir.dt.bfloat16
T = 1024
C = 64
NT = 8
P1 = 4096       # skew scratch row pitch (elements)
SCALE = 0.125   # 1/sqrt(C)
N_WARM = 4     # PE warm-up matmuls (HAM needs ~3.4us of sustained activity)
EXP = mybir.ActivationFunctionType.Exp


def rev_free(ap):
    """Reverse the (contiguous) free dim of a 2D AP."""
    (ps, pc), (fs, fc) = ap.ap
    assert fs == 1, ap.ap
    return AP(ap.tensor, ap.offset + (fc - 1), [[ps, pc], [-1, fc]])


def mm_chunks(lo, hi, step=512):
    a = lo
    while a < hi:
        b = min(hi, (a // step + 1) * step)
        yield a, b
        a = b


def emit(nc, tc, xta_d, ekr_d, const_d, yd):
    MULT = mybir.AluOpType.mult
    ADD = mybir.AluOpType.add
    with (
        tc.tile_pool(name="const", bufs=1) as cp,
        tc.tile_pool(name="work", bufs=1) as wp,
        tc.tile_pool(name="psum", bufs=1, space="PSUM") as pp,
        tc.tile_pool(name="dram", bufs=1, space="DRAM") as dp,
    ):
        QAD = dp.tile([T + 1, P1], BF, name="QAD").tensor

        # ---- loads ----
        XTA = cp.tile([65, T], BF)      # [x.T ; ones]
        KNE = cp.tile([128, T], BF)     # rows 0:64 k.T (natural); 64:128 embk.T-rev
        CONST = cp.tile([128, 768], BF)
        FILLC = cp.tile([128, 1024], BF)
        nc.vector.memset(FILLC, -4000.0)
        ZROW = cp.tile([1, 512], BF)
        nc.vector.memset(ZROW, 0.0)
        # XTA first (it gates qk); prefills next on the SAME queue as the
        # skew reads (sync) so FIFO order guarantees prefill-before-read.
        nc.sync.dma_start(out=XTA[0:33, :], in_=xta_d[0:33, :])
        nc.scalar.dma_start(out=XTA[33:65, :], in_=xta_d[33:65, :])
        nc.sync.dma_start(
            out=AP(QAD, P1 + 128, [[P1, 128], [128 * P1 + 128, NT], [1, 128]]),
            in_=FILLC.rearrange("p (b c) -> p b c", b=NT))
        nc.sync.dma_start(
            out=AP(QAD, P1 + 384, [[P1, 128], [128 * P1 + 256, NT], [1, 128]]),
            in_=FILLC.rearrange("p (b c) -> p b c", b=NT))
        nc.scalar.dma_start(out=KNE[64:128, :], in_=ekr_d[:, :])
        nc.gpsimd.dma_start(out=CONST[:, :], in_=const_d[:, :])
        EMBV = CONST[:, 0:512]          # embv row-packed [p, 64n+c]
        WVA = CONST[0:65, 512:576]      # [Wv.T ; bv]
        WQKB = CONST[0:65, 576:704]     # [[Wq.T | Wk.T] ; [bq | bk]]
        WPT = CONST[0:64, 704:768]      # Wproj.T

        identb = cp.tile([128, 128], BF)
        masks.make_identity(nc, identb)

        # ---- PE warm-up burst (garbage matmuls, result never read) ----
        wu = pp.tile([128, 512], F32, tag="A", bufs=4, name="wu")
        for _ in range(N_WARM):
            nc.tensor.matmul(wu[:, :], FILLC[:, 0:128], FILLC[:, 0:512],
                             start=True, stop=True)

        # ---- qk projection: [q.T ; k.T] = [Wq.T|Wk.T ; bq|bk].T @ [x.T ; 1]
        QT2 = cp.tile([128, T], BF)     # q.T duplicated in both halves
        for a, b in mm_chunks(0, T):
            ps_qk = pp.tile([128, 512], F32, tag="A" if a == 0 else "B",
                            bufs=4, name="ps_qk")
            nc.tensor.matmul(ps_qk[:, 0:b - a], WQKB, XTA[:, a:b],
                             start=True, stop=True)
            nc.scalar.copy(QT2[0:64, a:b], ps_qk[0:64, 0:b - a])
            nc.vector.tensor_copy(QT2[64:128, a:b], ps_qk[0:64, 0:b - a])
            nc.vector.tensor_copy(KNE[0:64, a:b], ps_qk[64:128, 0:b - a])

        Zc = cp.tile([128, NT], F32)
        rz = cp.tile([128, NT], F32)

        # ---- stage 1: score matmuls (row-packed pairs), PSUM evac, skew ----
        qa_t = {}
        au2_t = {}

        def stage1(i):
            Wd = 128 * (i + 1)
            i0 = 128 * i
            qa = cp.tile([128, 2048], BF, tag=f"qa{i}", name=f"qa{i}")
            qa_t[i] = qa
            for a, b in mm_chunks(0, Wd):
                w = b - a
                ps_a1 = pp.tile([128, 512], F32, tag="A", bufs=4, name="ps_a1")
                ps_qe = pp.tile([128, 512], F32, tag="B", bufs=4, name="ps_qe")
                nc.tensor.matmul(ps_a1[:, 0:w], QT2[0:64, i0:i0 + 128],
                                 KNE[0:64, a:b],
                                 start=True, stop=True, tile_position=(0, 0))
                nc.tensor.matmul(ps_qe[:, 0:w], QT2[64:128, i0:i0 + 128],
                                 KNE[64:128, T - Wd + a:T - Wd + b],
                                 start=True, stop=True, tile_position=(64, 0))
                # qa row = [qe-rev (Wd) | a1-rev (Wd)]; DVE/ACT split
                nc.vector.tensor_copy(qa[:, a:b], ps_qe[:, 0:w])
                nc.scalar.copy(
                    rev_free(qa[:, 2 * Wd - b:2 * Wd - a]), ps_a1[:, 0:w])
            for a, b in mm_chunks(0, Wd):
                w = b - a
                nc.gpsimd.dma_start(
                    out=AP(QAD, (i0 + 1) * P1 + a,
                           [[P1, 128], [2 * Wd + 128 - a - b, 2], [1, w]]),
                    in_=AP(qa[:, :].tensor, qa[:, :].offset + a,
                           [[qa[:, :].ap[0][0], 128], [2 * Wd - a - b, 2], [1, w]]))
            # merged skew read: a2 = [:, 0:Wd], a1U = [:, Wd+128:2Wd+128]
            L = 2 * Wd + 128
            au2 = cp.tile([128, 2304], BF, tag=f"au2{i}", name=f"au2{i}")
            au2_t[i] = au2
            nc.sync.dma_start(
                out=au2[:, 0:L],
                in_=AP(QAD, (i0 + 1) * P1 + 127, [[P1 - 1, 128], [1, L]]))

        V = cp.tile([128, 512], BF)     # v[128n+p, c] at [p, 64n+c]
        ETB = cp.tile([128, NT * 1024], BF, name="ETB")
        AUTB = cp.tile([128, NT * 1024], BF, name="AUTB")
        ETB3 = ETB.rearrange("p (k c) -> p k c", c=1024)
        AUTB3 = AUTB.rearrange("p (k c) -> p k c", c=1024)
        ps_y1 = pp.tile([C, 512], F32, tag="A", bufs=4, name="ps_y1")
        ps_y0 = pp.tile([C, 512], F32, tag="B", bufs=4, name="ps_y0")
        enau_t = {}

        def stage2(i):
            Wd = 128 * (i + 1)
            qa = qa_t[i]
            au2 = au2_t[i]
            s1 = wp.tile([128, 2048], BF, tag="s1", bufs=3)
            nc.vector.scalar_tensor_tensor(
                out=s1[:, 0:Wd], in0=rev_free(qa[:, Wd:2 * Wd]), scalar=1.0,
                in1=au2[:, 0:Wd], op0=MULT, op1=ADD)
            nc.vector.scalar_tensor_tensor(
                out=s1[:, 1024:1024 + Wd], in0=rev_free(qa[:, 0:Wd]),
                scalar=1.0, in1=au2[:, Wd + 128:2 * Wd + 128],
                op0=MULT, op1=ADD)
            enau = cp.tile([128, 2048], BF, tag=f"enau{i}", name=f"enau{i}")
            enau_t[i] = enau
            nc.scalar.activation(enau[:, 0:Wd], s1[:, 0:Wd], EXP, scale=SCALE,
                                 accum_out=Zc[:, i:i + 1])
            nc.scalar.activation(enau[:, 1024:1024 + Wd],
                                 s1[:, 1024:1024 + Wd], EXP, scale=SCALE)

        def stage3(i):
            i0 = 128 * i
            enau = enau_t[i]
            for half, dst3 in ((0, ETB3), (1, AUTB3)):
                for g in range(0, i + 1, 4):
                    gsz = min(4, i + 1 - g)
                    ps_t4 = pp.tile([128, 512], BF, tag="B", bufs=4,
                                    name="ps_t4")
                    for j in range(gsz):
                        k = g + j
                        nc.tensor.matmul(
                            ps_t4[:, 128 * j:128 * (j + 1)],
                            enau[:, 1024 * half + 128 * k:1024 * half + 128 * (k + 1)],
                            identb[:, :], is_transpose=True,
                            start=(j == 0), stop=(j == gsz - 1))
                    cpy = nc.vector.tensor_copy if (i + g // 4 + half) % 2 \
                        else nc.scalar.copy
                    cpy(dst3[:, g:g + gsz, i0:i0 + 128],
                        ps_t4[:, 0:128 * gsz].rearrange("p (k c) -> p k c", c=128))
            # value terms: processing order 4,5,6,7,3,2,1,0 -- after tile 7
            # all of k=7..4 are ready; smaller k become ready one per tile
            for k in ([7, 6, 5, 4, 3] if i == 7 else [i] if i < 3 else []):
                k0 = 128 * k
                ta = max(512, k0)
                nc.tensor.matmul(ps_y1[:, ta - 512:512], V[:, C * k:C * (k + 1)],
                                 ETB[:, 1024 * k + ta:1024 * k + T],
                                 start=False, stop=False)
                nc.tensor.matmul(ps_y1[:, ta - 512:512],
                                 EMBV[:, C * k:C * (k + 1)],
                                 AUTB[:, 1024 * k + ta:1024 * k + T],
                                 start=False, stop=(k == 0))
                if k0 < 512:
                    nc.tensor.matmul(ps_y0[:, k0:512], V[:, C * k:C * (k + 1)],
                                     ETB[:, 1024 * k + k0:1024 * k + 512],
                                     start=False, stop=False)
                    nc.tensor.matmul(ps_y0[:, k0:512],
                                     EMBV[:, C * k:C * (k + 1)],
                                     AUTB[:, 1024 * k + k0:1024 * k + 512],
                                     start=False, stop=(k == 0))

        # ---- v4 driver: stage1 all, v-projs, then lag-1 stage2/stage3 ----
        ORDER = [3, 4, 5, 6, 7, 2, 1, 0]
        for i in ORDER:
            stage1(i)
        # v projection (PE filler during the first tiles' skew round trips)
        for n in range(NT):
            ps_v = pp.tile([128, C], F32, tag="A", bufs=4, name="ps_v")
            nc.tensor.matmul(ps_v[:, :], XTA[:, 128 * n:128 * (n + 1)], WVA,
                             start=True, stop=True)
            nc.vector.tensor_copy(V[:, C * n:C * (n + 1)], ps_v[:, :])
        nc.tensor.matmul(ps_y1[:, :], ZROW[:, 0:C], ZROW[:, :],
                         start=True, stop=False)
        nc.tensor.matmul(ps_y0[:, :], ZROW[:, 0:C], ZROW[:, :],
                         start=True, stop=False)
        prev = None
        for i in ORDER:
            stage2(i)
            if prev is not None:
                stage3(prev)
            prev = i
        nc.vector.reciprocal(rz[:, :], Zc[:, :])
        stage3(0)

        YSB = cp.tile([C, T], BF)
        nc.vector.tensor_copy(YSB[:, 512:1024], ps_y1[:, :])
        nc.vector.tensor_copy(YSB[:, 0:512], ps_y0[:, :])

        # ---- output projection + 1/Z; one batched output DMA ----
        YQ = cp.tile([128, 512], F32)
        for i in range(NT):
            ps_p = pp.tile([128, C], F32, tag="A", bufs=4, name="ps_p")
            nc.tensor.matmul(ps_p[:, :], YSB[:, 128 * i:128 * (i + 1)], WPT,
                             start=True, stop=True)
            nc.vector.tensor_scalar_mul(YQ[:, C * i:C * (i + 1)], ps_p[:, :],
                                        rz[:, i:i + 1])
        nc.gpsimd.dma_start(
            out=AP(yd.tensor, yd.offset, [[C, 128], [128 * C, NT], [1, C]]),
            in_=YQ.rearrange("p (i c) -> p i c", c=C))


_NC_CACHE = None


def _build():
    global _NC_CACHE
    if _NC_CACHE is not None:
        return _NC_CACHE
    nc = bacc.Bacc("TRN2", target_bir_lowering=False, debug=False)
    xta_d = nc.dram_tensor("xta", [65, T], BF, kind="ExternalInput")
    ekr_d = nc.dram_tensor("ekr", [C, T], BF, kind="ExternalInput")
    const_d = nc.dram_tensor("cpack", [128, 768], BF, kind="ExternalInput")
    yd = nc.dram_tensor("y", [T, C], F32, kind="ExternalOutput")
    from concourse.tile import TileContext
    with TileContext(nc) as tc:
        emit(nc, tc, xta_d.ap(), ekr_d.ap(), const_d.ap(), yd.ap())
    nc.compile()
    _NC_CACHE = nc
    return nc


def _prep(inputs):
    """Host-side packing of all device inputs (bf16, pre-transposed)."""
    bf = ml_dtypes.bfloat16
    x = np.asarray(inputs["x"], dtype=np.float32)
    Wqkv = np.asarray(inputs["Wqkv"], dtype=np.float32)
    bqkv = np.asarray(inputs["bqkv"], dtype=np.float32)
    embk = np.asarray(inputs["embk"], dtype=np.float32)
    embv = np.asarray(inputs["embv"], dtype=np.float32)
    Wproj = np.asarray(inputs["Wproj"], dtype=np.float32)

    B = x.shape[0]
    xta = np.empty((B, 65, T), dtype=bf)
    for b in range(B):
        xta[b, 0:64] = x[b].T.astype(bf)
        xta[b, 64] = 1.0
    ekr = np.ascontiguousarray(embk.T[:, ::-1]).astype(bf)

    const = np.zeros((128, 768), dtype=bf)
    const[:, 0:512] = embv.reshape(8, 128, 64).transpose(1, 0, 2).reshape(128, 512).astype(bf)
    const[0:64, 512:576] = Wqkv[128:192, :].T.astype(bf)
    const[64, 512:576] = bqkv[128:192].astype(bf)
    const[0:64, 576:704] = Wqkv[0:128, :].T.astype(bf)
    const[64, 576:704] = bqkv[0:128].astype(bf)
    const[0:64, 704:768] = Wproj.T.astype(bf)
    return xta, np.ascontiguousarray(ekr), np.ascontiguousarray(const)


def run_spmd(inputs, **kwargs):
    from concourse.bass_utils import run_bass_kernel_spmd
    x = np.asarray(inputs["x"], dtype=np.float32)
    B = x.shape[0]
    nc = _build()
    xta, ekr, const = _prep(inputs)
    in_maps = [dict(xta=np.ascontiguousarray(xta[b]), ekr=ekr, cpack=const)
               for b in range(B)]
    res = run_bass_kernel_spmd(nc, in_maps, core_ids=list(range(B)), **kwargs)
    bproj = np.asarray(inputs["bproj"], dtype=np.float32)
    y = np.stack([r["y"] for r in res.results], axis=0) + bproj[None, None, :]
    return y, res


def kernel(**inputs):
    y, _ = run_spmd(inputs)
    return y


# revision 47
# speedup vs baseline: 1.2219x; 1.1867x over previous
"""Trainium2 Bass kernel for nn_CausalSelfAttention_2783138808334.

B=8, T=1024, C=64, n_head=1. Data-parallel over batch: one batch per
NeuronCore across 8 cores (weights/tables replicated), gathered on the host.

Per-core algorithm:
  qkv = x @ Wqkv.T + b (bias folded in via an augmented K=65 contraction with
  a host-provided ones row); causal attention with relative-position tables;
  y = (att @ v + attU @ embv) @ Wproj.T (+ bproj added on the host).

Relative attention is computed in TWO domains concurrently:
  s-domain:  att[t,s] = a1[t,s] + QE[t,t-s]      (QE = q @ embk.T)
  u-domain: attU[t,u] = a1[t,t-u] + QE[t,u]
so the two diagonal "skews" (of QE and of a1) are independent and overlap.
Each skew writes REVERSED rows to a DRAM scratch at pitch P1 and reads back
with partition step P1-1 (unit inner stride). Both matrices ride ONE scratch
row per t -- [qe-rev | -4000 gap | a1-rev | -4000 gap] -- so one write + one
read per tile covers both domains, and the prefilled -4000 gaps land exactly
on the causal-mask region (exp -> 0), eliminating all masking ops.

E / AU are transposed 128x128-blockwise on the TensorEngine (groups of 4
into one PSUM bank, one strided copy per group into the big ETB/AUTB tiles).
Tiles are processed in order 4,5,6,7,3,2,1,0 so a medium tile's skew round
trip completes first and bridges the PE from the score matmuls into the
transpose+value phase; value term k becomes ready right after tile k's
transposes. Score matmul pairs run concurrently on PE row-groups (0,0)/(64,0)
via tile_position. A warm-up burst flips the HAM clock gate to 8/8 early.
"""
import numpy as np
import ml_dtypes

import concourse.bass as bass
import concourse.bacc as bacc
import concourse.mybir as mybir
from concourse import masks
from concourse.ap import AP

F32 = mybir.dt.float32
BF = mybir.dt.bfloat16
T = 1024
C = 64
NT = 8
P1 = 4096       # skew scratch row pitch (elements)
SCALE = 0.125   # 1/sqrt(C)
N_WARM = 4     # PE warm-up matmuls (HAM needs ~3.4us of sustained activity)
EXP = mybir.ActivationFunctionType.Exp


def rev_free(ap):
    """Reverse the (contiguous) free dim of a 2D AP."""
    (ps, pc), (fs, fc) = ap.ap
    assert fs == 1, ap.ap
    return AP(ap.tensor, ap.offset + (fc - 1), [[ps, pc], [-1, fc]])


def mm_chunks(lo, hi, step=512):
    a = lo
    while a < hi:
        b = min(hi, (a // step + 1) * step)
        yield a, b
        a = b


def emit(nc, tc, xta_d, ekr_d, const_d, yd):
    MULT = mybir.AluOpType.mult
    ADD = mybir.AluOpType.add
    with (
        tc.tile_pool(name="const", bufs=1) as cp,
        tc.tile_pool(name="work", bufs=1) as wp,
        tc.tile_pool(name="psum", bufs=1, space="PSUM") as pp,
        tc.tile_pool(name="dram", bufs=1, space="DRAM") as dp,
    ):
        QAD = dp.tile([T + 1, P1], BF, name="QAD").tensor

        # ---- loads ----
        XTA = cp.tile([65, T], BF)      # [x.T ; ones]
        KNE = cp.tile([128, T], BF)     # rows 0:64 k.T (natural); 64:128 embk.T-rev
        CONST = cp.tile([128, 768], BF)
        FILLC = cp.tile([128, 1024], BF)
        nc.vector.memset(FILLC, -4000.0)
        ZROW = cp.tile([1, 512], BF)
        nc.vector.memset(ZROW, 0.0)
        # XTA first (it gates qk); prefills next on the SAME queue as the
        # skew reads (sync) so FIFO order guarantees prefill-before-read.
        nc.sync.dma_start(out=XTA[0:33, :], in_=xta_d[0:33, :])
        nc.scalar.dma_start(out=XTA[33:65, :], in_=xta_d[33:65, :])
        nc.sync.dma_start(
            out=AP(QAD, P1 + 128, [[P1, 128], [128 * P1 + 128, NT], [1, 128]]),
            in_=FILLC.rearrange("p (b c) -> p b c", b=NT))
        nc.sync.dma_start(
            out=AP(QAD, P1 + 384, [[P1, 128], [128 * P1 + 256, NT], [1, 128]]),
            in_=FILLC.rearrange("p (b c) -> p b c", b=NT))
        nc.scalar.dma_start(out=KNE[64:128, :], in_=ekr_d[:, :])
        nc.gpsimd.dma_start(out=CONST[:, :], in_=const_d[:, :])
        EMBV = CONST[:, 0:512]          # embv row-packed [p, 64n+c]
        WVA = CONST[0:65, 512:576]      # [Wv.T ; bv]
        WQKB = CONST[0:65, 576:704]     # [[Wq.T | Wk.T] ; [bq | bk]]
        WPT = CONST[0:64, 704:768]      # Wproj.T

        identb = cp.tile([128, 128], BF)
        masks.make_identity(nc, identb)

        # ---- PE warm-up burst (garbage matmuls, result never read) ----
        wu = pp.tile([128, 512], F32, tag="A", bufs=4, name="wu")
        for _ in range(N_WARM):
            nc.tensor.matmul(wu[:, :], FILLC[:, 0:128], FILLC[:, 0:512],
                             start=True, stop=True)

        # ---- qk projection: [q.T ; k.T] = [Wq.T|Wk.T ; bq|bk].T @ [x.T ; 1]
        QT2 = cp.tile([128, T], BF)     # q.T duplicated in both halves
        for a, b in mm_chunks(0, T):
            ps_qk = pp.tile([128, 512], F32, tag="A" if a == 0 else "B",
                            bufs=4, name="ps_qk")
            nc.tensor.matmul(ps_qk[:, 0:b - a], WQKB, XTA[:, a:b],
                             start=True, stop=True)
            nc.scalar.copy(QT2[0:64, a:b], ps_qk[0:64, 0:b - a])
            nc.vector.tensor_copy(QT2[64:128, a:b], ps_qk[0:64, 0:b - a])
            nc.vector.tensor_copy(KNE[0:64, a:b], ps_qk[64:128, 0:b - a])

        Zc = cp.tile([128, NT], F32)
        rz = cp.tile([128, NT], F32)

        # ---- stage 1: score matmuls (row-packed pairs), PSUM evac, skew ----
        qa_t = {}
        au2_t = {}

        def stage1(i):
            Wd = 128 * (i + 1)
            i0 = 128 * i
            qa = cp.tile([128, 2048], BF, tag=f"qa{i}", name=f"qa{i}")
            qa_t[i] = qa
            for a, b in mm_chunks(0, Wd):
                w = b - a
                ps_a1 = pp.tile([128, 512], F32, tag="A", bufs=4, name="ps_a1")
                ps_qe = pp.tile([128, 512], F32, tag="B", bufs=4, name="ps_qe")
                nc.tensor.matmul(ps_a1[:, 0:w], QT2[0:64, i0:i0 + 128],
                                 KNE[0:64, a:b],
                                 start=True, stop=True, tile_position=(0, 0))
                nc.tensor.matmul(ps_qe[:, 0:w], QT2[64:128, i0:i0 + 128],
                                 KNE[64:128, T - Wd + a:T - Wd + b],
                                 start=True, stop=True, tile_position=(64, 0))
                # qa row = [qe-rev (Wd) | a1-rev (Wd)]; DVE/ACT split
                nc.vector.tensor_copy(qa[:, a:b], ps_qe[:, 0:w])
                nc.scalar.copy(
                    rev_free(qa[:, 2 * Wd - b:2 * Wd - a]), ps_a1[:, 0:w])
            for a, b in mm_chunks(0, Wd):
                w = b - a
                nc.gpsimd.dma_start(
                    out=AP(QAD, (i0 + 1) * P1 + a,
                           [[P1, 128], [2 * Wd + 128 - a - b, 2], [1, w]]),
                    in_=AP(qa[:, :].tensor, qa[:, :].offset + a,
                           [[qa[:, :].ap[0][0], 128], [2 * Wd - a - b, 2], [1, w]]))
            # merged skew read: a2 = [:, 0:Wd], a1U = [:, Wd+128:2Wd+128]
            L = 2 * Wd + 128
            au2 = cp.tile([128, 2304], BF, tag=f"au2{i}", name=f"au2{i}")
            au2_t[i] = au2
            nc.sync.dma_start(
                out=au2[:, 0:L],
                in_=AP(QAD, (i0 + 1) * P1 + 127, [[P1 - 1, 128], [1, L]]))

        V = cp.tile([128, 512], BF)     # v[128n+p, c] at [p, 64n+c]
        ETB = cp.tile([128, NT * 1024], BF, name="ETB")
        AUTB = cp.tile([128, NT * 1024], BF, name="AUTB")
        ETB3 = ETB.rearrange("p (k c) -> p k c", c=1024)
        AUTB3 = AUTB.rearrange("p (k c) -> p k c", c=1024)
        ps_y1 = pp.tile([C, 512], F32, tag="A", bufs=4, name="ps_y1")
        ps_y0 = pp.tile([C, 512], F32, tag="B", bufs=4, name="ps_y0")
        enau_t = {}

        def stage2(i):
            Wd = 128 * (i + 1)
            qa = qa_t[i]
            au2 = au2_t[i]
            s1 = wp.tile([128, 2048], BF, tag="s1", bufs=3)
            nc.vector.scalar_tensor_tensor(
                out=s1[:, 0:Wd], in0=rev_free(qa[:, Wd:2 * Wd]), scalar=1.0,
                in1=au2[:, 0:Wd], op0=MULT, op1=ADD)
            nc.vector.scalar_tensor_tensor(
                out=s1[:, 1024:1024 + Wd], in0=rev_free(qa[:, 0:Wd]),
                scalar=1.0, in1=au2[:, Wd + 128:2 * Wd + 128],
                op0=MULT, op1=ADD)
            enau = cp.tile([128, 2048], BF, tag=f"enau{i}", name=f"enau{i}")
            enau_t[i] = enau
            nc.scalar.activation(enau[:, 0:Wd], s1[:, 0:Wd], EXP, scale=SCALE,
                                 accum_out=Zc[:, i:i + 1])
            nc.scalar.activation(enau[:, 1024:1024 + Wd],
                                 s1[:, 1024:1024 + Wd], EXP, scale=SCALE)

        def stage3(i):
            i0 = 128 * i
            enau = enau_t[i]
            for half, dst3 in ((0, ETB3), (1, AUTB3)):
                for g in range(0, i + 1, 8):
                    gsz = min(8, i + 1 - g)
                    ps_t4 = pp.tile([128, 1024], BF, tag="B", bufs=4,
                                    name="ps_t4")
                    for j in range(gsz):
                        k = g + j
                        nc.tensor.matmul(
                            ps_t4[:, 128 * j:128 * (j + 1)],
                            enau[:, 1024 * half + 128 * k:1024 * half + 128 * (k + 1)],
                            identb[:, :], is_transpose=True,
                            start=(j == 0), stop=(j == gsz - 1))
                    cpy = nc.vector.tensor_copy if (i + g // 4 + half) % 2 \
                        else nc.scalar.copy
                    cpy(dst3[:, g:g + gsz, i0:i0 + 128],
                        ps_t4[:, 0:128 * gsz].rearrange("p (k c) -> p k c", c=128))
            # value terms: processing order 4,5,6,7,3,2,1,0 -- after tile 7
            # all of k=7..4 are ready; smaller k become ready one per tile
            for k in ([7, 6, 5, 4] if i == 7 else [i] if i < 4 else []):
                k0 = 128 * k
                ta = max(512, k0)
                nc.tensor.matmul(ps_y1[:, ta - 512:512], V[:, C * k:C * (k + 1)],
                                 ETB[:, 1024 * k + ta:1024 * k + T],
                                 start=False, stop=False)
                nc.tensor.matmul(ps_y1[:, ta - 512:512],
                                 EMBV[:, C * k:C * (k + 1)],
                                 AUTB[:, 1024 * k + ta:1024 * k + T],
                                 start=False, stop=(k == 0))
                if k0 < 512:
                    nc.tensor.matmul(ps_y0[:, k0:512], V[:, C * k:C * (k + 1)],
                                     ETB[:, 1024 * k + k0:1024 * k + 512],
                                     start=False, stop=False)
                    nc.tensor.matmul(ps_y0[:, k0:512],
                                     EMBV[:, C * k:C * (k + 1)],
                                     AUTB[:, 1024 * k + k0:1024 * k + 512],
                                     start=False, stop=(k == 0))

        # ---- v4 driver: stage1 all, v-projs, then lag-1 stage2/stage3 ----
        ORDER = [4, 5, 6, 7, 3, 2, 1, 0]
        for i in ORDER:
            stage1(i)
        # v projection (PE filler during the first tiles' skew round trips)
        for n in range(NT):
            ps_v = pp.tile([128, C], F32, tag="A", bufs=4, name="ps_v")
            nc.tensor.matmul(ps_v[:, :], XTA[:, 128 * n:128 * (n + 1)], WVA,
                             start=True, stop=True)
            nc.vector.tensor_copy(V[:, C * n:C * (n + 1)], ps_v[:, :])
        nc.tensor.matmul(ps_y1[:, :], ZROW[:, 0:C], ZROW[:, :],
                         start=True, stop=False)
        nc.tensor.matmul(ps_y0[:, :], ZROW[:, 0:C], ZROW[:, :],
                         start=True, stop=False)
        prev = None
        for i in ORDER:
            stage2(i)
            if prev is not None:
                stage3(prev)
            prev = i
        nc.vector.reciprocal(rz[:, :], Zc[:, :])
        stage3(0)

        YSB = cp.tile([C, T], BF)
        nc.vector.tensor_copy(YSB[:, 512:1024], ps_y1[:, :])
        nc.vector.tensor_copy(YSB[:, 0:512], ps_y0[:, :])

        # ---- output projection + 1/Z; one batched output DMA ----
        YQ = cp.tile([128, 512], F32)
        for i in range(NT):
            ps_p = pp.tile([128, C], F32, tag="A", bufs=4, name="ps_p")
            nc.tensor.matmul(ps_p[:, :], YSB[:, 128 * i:128 * (i + 1)], WPT,
                             start=True, stop=True)
            nc.vector.tensor_scalar_mul(YQ[:, C * i:C * (i + 1)], ps_p[:, :],
                                        rz[:, i:i + 1])
        nc.gpsimd.dma_start(
            out=AP(yd.tensor, yd.offset, [[C, 128], [128 * C, NT], [1, C]]),
            in_=YQ.rearrange("p (i c) -> p i c", c=C))


_NC_CACHE = None


def _build():
    global _NC_CACHE
    if _NC_CACHE is not None:
        return _NC_CACHE
    nc = bacc.Bacc("TRN2", target_bir_lowering=False, debug=False)
    xta_d = nc.dram_tensor("xta", [65, T], BF, kind="ExternalInput")
    ekr_d = nc.dram_tensor("ekr", [C, T], BF, kind="ExternalInput")
    const_d = nc.dram_tensor("cpack", [128, 768], BF, kind="ExternalInput")
    yd = nc.dram_tensor("y", [T, C], F32, kind="ExternalOutput")
    from concourse.tile import TileContext
    with TileContext(nc) as tc:
        emit(nc, tc, xta_d.ap(), ekr_d.ap(), const_d.ap(), yd.ap())
    nc.compile()
    _NC_CACHE = nc
    return nc


def _prep(inputs):
    """Host-side packing of all device inputs (bf16, pre-transposed)."""
    bf = ml_dtypes.bfloat16
    x = np.asarray(inputs["x"], dtype=np.float32)
    Wqkv = np.asarray(inputs["Wqkv"], dtype=np.float32)
    bqkv = np.asarray(inputs["bqkv"], dtype=np.float32)
    embk = np.asarray(inputs["embk"], dtype=np.float32)
    embv = np.asarray(inputs["embv"], dtype=np.float32)
    Wproj = np.asarray(inputs["Wproj"], dtype=np.float32)

    B = x.shape[0]
    xta = np.empty((B, 65, T), dtype=bf)
    for b in range(B):
        xta[b, 0:64] = x[b].T.astype(bf)
        xta[b, 64] = 1.0
    ekr = np.ascontiguousarray(embk.T[:, ::-1]).astype(bf)

    const = np.zeros((128, 768), dtype=bf)
    const[:, 0:512] = embv.reshape(8, 128, 64).transpose(1, 0, 2).reshape(128, 512).astype(bf)
    const[0:64, 512:576] = Wqkv[128:192, :].T.astype(bf)
    const[64, 512:576] = bqkv[128:192].astype(bf)
    const[0:64, 576:704] = Wqkv[0:128, :].T.astype(bf)
    const[64, 576:704] = bqkv[0:128].astype(bf)
    const[0:64, 704:768] = Wproj.T.astype(bf)
    return xta, np.ascontiguousarray(ekr), np.ascontiguousarray(const)


def run_spmd(inputs, **kwargs):
    from concourse.bass_utils import run_bass_kernel_spmd
    x = np.asarray(inputs["x"], dtype=np.float32)
    B = x.shape[0]
    nc = _build()
    xta, ekr, const = _prep(inputs)
    in_maps = [dict(xta=np.ascontiguousarray(xta[b]), ekr=ekr, cpack=const)
               for b in range(B)]
    res = run_bass_kernel_spmd(nc, in_maps, core_ids=list(range(B)), **kwargs)
    bproj = np.asarray(inputs["bproj"], dtype=np.float32)
    y = np.stack([r["y"] for r in res.results], axis=0) + bproj[None, None, :]
    return y, res


def kernel(**inputs):
    y, _ = run_spmd(inputs)
    return y
